# revision 23
# baseline (speedup 1.0000x reference)
"""Trainium2 Bass kernel for nn_CRnnGenerator (8-core SPMD).

Math restructuring (exact, up to float rounding):
  reference per-step:  x_t = relu([z_t, pg_t] @ fc1_w.T + fc1_b)
                       (h1,c1) = LSTM1(x_t, h1, c1); (h2,c2) = LSTM2(h1, h2, c2)
                       pg_{t+1} = h2 @ fc2_w.T + fc2_b ;  outs[t] = pg_{t+1}
  Split fc1_w = [W1z | W1p].  Then
     z_t @ W1z.T        -> Zp = z_flat @ W1z.T          (one big matmul, upfront)
     pg_t @ W1p.T       -> for t>=1: h2_t @ (W1p@fc2_w).T + W1p@fc2_b
                           where M := W1p @ fc2_w is only [128,128]
                        -> for t=0:  prev_gen @ W1p.T    (one matvec, upfront)
     outs               -> H2 @ fc2_w.T + fc2_b          (one big matmul, at end)
  So the serial 64-step loop touches only 128-dim tensors; all large traffic
  (fc1_w 201MB, fc2_w 100MB, z/y 50MB each) is read exactly once, sharded
  8 ways over the feature dimension with a single ~100KB AllReduce.
"""

import numpy as np

import concourse.bass as bass
import concourse.bacc as bacc
import concourse.tile as tile
import concourse.mybir as mybir
from concourse import bass_utils
from concourse.alu_op_type import AluOpType
from concourse.mybir import ActivationFunctionType as AF

F16 = mybir.dt.float16
F32 = mybir.dt.float32

S = 64
NF = 196608
HD = 128
NCORES = 8
KS = NF // NCORES          # 24576 contraction elems per core
NCH = KS // 128            # 192 k-chunks of 128
QG = 4                     # k-chunks per DMA group
NQ = NCH // QG             # 48 DMA groups
ND = KS // 512             # 48 output chunks of 512
CW = 450                   # per-chunk packed width: w1z|w1p|fc2|z|vec
GQ = 8                     # chunks per P1 DMA

# torch LSTM gate order in weight rows: i, f, g, o.  We want [f, i, o, g].
_GIDX = np.r_[128:256, 0:128, 384:512, 256:384]


def _emit(nc, tc, t_in, t_out, ncores=NCORES, phases="all"):
    bigin, fc2t, ypb = (t_in[k] for k in ("bigin", "fc2t", "ypb"))
    wih1t, whh1t, wih2t, whh2t = (
        t_in[k] for k in ("wih1t", "whh1t", "wih2t", "whh2t"))
    b1c, b2c, fc1b, h016, c0in = (
        t_in[k] for k in ("b1c", "b2c", "fc1b", "h016", "c0in"))
    outp, stout = t_out["outp"], t_out["stout"]

    mm = nc.tensor.matmul
    act = nc.scalar.activation
    dve = nc.vector

    with tc.tile_pool(name="pers", bufs=1) as pers:
        # ---- persistent SBUF tiles ---------------------------------------
        cw1 = pers.tile([128, 512], F16, tag="cw1")       # wih1t
        cw2 = pers.tile([128, 512], F16, tag="cw2")       # whh1t
        cw3 = pers.tile([128, 512], F16, tag="cw3")       # wih2t
        cw4 = pers.tile([128, 512], F16, tag="cw4")       # whh2t
        cb1 = pers.tile([128, 4], F32, tag="cb1")
        cb2 = pers.tile([128, 4], F32, tag="cb2")
        cfb = pers.tile([128, 1], F32, tag="cfb")
        ch0 = pers.tile([128, 2], F16, tag="ch0")
        cc0 = pers.tile([128, 2], F32, tag="cc0")
        nc.sync.dma_start(cw1[:], wih1t.ap())
        nc.sync.dma_start(cw2[:], whh1t.ap())
        nc.sync.dma_start(cw3[:], wih2t.ap())
        nc.sync.dma_start(cw4[:], whh2t.ap())
        nc.sync.dma_start(cb1[:], b1c.ap())
        nc.sync.dma_start(cb2[:], b2c.ap())
        nc.sync.dma_start(cfb[:], fc1b.ap())
        nc.sync.dma_start(ch0[:], h016.ap())
        nc.sync.dma_start(cc0[:], c0in.ap())

        H1T = pers.tile([128, S], F16, tag="H1T")         # h1 after each step
        H2T = pers.tile([128, S], F16, tag="H2T")         # h2 after each step
        C1 = pers.tile([128, 1], F32, tag="C1")
        C2 = pers.tile([128, 1], F32, tag="C2")
        Rt = pers.tile([128, 194], F32, tag="Rt")         # reduced [ZpT|MT|v|pg0p]
        MT16 = pers.tile([128, 128], F16, tag="MT16")
        ZpB = pers.tile([128, S], F32, tag="ZpB")
        bbv = pers.tile([128, 1], F32, tag="bbv")         # v + fc1_b
        t0b = pers.tile([128, 1], F32, tag="t0b")         # pg0p + fc1_b
        so = pers.tile([128, 4], F32, tag="so")           # final states

        # ---- phase 1: partial projections, K-sharded ----------------------
        # bigin[p, m, 0:450] = [w1z(128) | w1p(128) | fc2(128) | z(64) | vec(2)]
        # per k-chunk m; per-partition runs are GQ*900B contiguous per DMA.
        big_g = bigin.ap()
        with (
            tc.tile_pool(name="pacc", bufs=1, space="PSUM") as pacc,
            tc.tile_pool(name="p1", bufs=3) as p1,
            tc.tile_pool(name="drp", bufs=1, space="DRAM") as drp,
        ):
            zp_ps = pacc.tile([128, S], F32, tag="zp")
            mt_ps = pacc.tile([128, 128], F32, tag="mt")
            vp_ps = pacc.tile([128, 2], F32, tag="vp")

            for q in range(NCH // GQ):
                tb = p1.tile([128, GQ * CW], F16, tag="big")
                nc.sync.dma_start(
                    tb[:].rearrange("p (c w) -> p c w", c=GQ),
                    big_g[:, q * GQ:(q + 1) * GQ])
                for c in range(GQ):
                    m = q * GQ + c
                    st = (m == 0)
                    sp = (m == NCH - 1)
                    o = c * CW
                    w1z_s = tb[:, o:o + 128]
                    w1p_s = tb[:, o + 128:o + 256]
                    fc2_s = tb[:, o + 256:o + 384]
                    z_s = tb[:, o + 384:o + 448]
                    vec_s = tb[:, o + 448:o + 450]
                    # ZpT[j,t] += sum_k w1zT[k,j] zT[k,t]
                    mm(zp_ps[:], w1z_s, z_s, start=st, stop=sp)
                    # MT[j,i] += sum_k fc2[k,j] w1pT[k,i]  ( = (W1p@fc2).T )
                    mm(mt_ps[:], fc2_s, w1p_s, start=st, stop=sp)
                    # vp[:,0] += w1pT.T fc2_b ; vp[:,1] += w1pT.T prev_gen
                    mm(vp_ps[:], w1p_s, vec_s, start=st, stop=sp)

            # ---- phase 2: AllReduce of [ZpT | MT | v | pg0p] --------------
            pk = p1.tile([128, 194], F32, tag="pk", bufs=1)
            dve.tensor_copy(pk[:, 0:S], zp_ps[:])
            dve.tensor_copy(pk[:, S:S + 128], mt_ps[:])
            dve.tensor_copy(pk[:, S + 128:194], vp_ps[:])
            if ncores == 1:
                dve.tensor_copy(Rt[:], pk[:])
            else:
                arin = drp.tile([128, 194], F32, tag="arin")
                arout = drp.tile([128, 194], F32, tag="arout")
                nc.sync.dma_start(arin[:], pk[:])
                nc.gpsimd.collective_compute(
                    "AllReduce", AluOpType.add,
                    replica_groups=[list(range(ncores))],
                    ins=[arin.opt()], outs=[arout.opt()])
                nc.sync.dma_start(Rt[:], arout[:])

        # ---- phase 3: recurrence setup -----------------------------------
        dve.tensor_copy(MT16[:], Rt[:, S:S + 128])
        dve.tensor_add(bbv[:], Rt[:, S + 128:S + 129], cfb[:])
        dve.tensor_add(t0b[:], Rt[:, S + 129:S + 130], cfb[:])
        dve.tensor_scalar_add(ZpB[:], Rt[:, 0:S], bbv[:])
        dve.tensor_copy(C1[:], cc0[:, 0:1])
        dve.tensor_copy(C2[:], cc0[:, 1:2])

        # ---- phase 4: 64 serial steps (128-dim only) ----------------------
        if phases == "p1":
            dve.memset(so[:], 0.0)
            nc.sync.dma_start(stout.ap(), so[:])
            return
        # prefetch phase-5 inputs now; their DMAs have no deps on the
        # recurrence so they overlap it
        NP = ND // 2
        fts, yts = [], []
        if phases == "all":
            pf_cm = tc.tile_pool(name="pf", bufs=NP)
            pf = pf_cm.__enter__()
            py_cm = tc.tile_pool(name="py", bufs=NP)
            py = py_cm.__enter__()
            ypb_g = ypb.ap().rearrange("t (e dd n) -> e dd t n", dd=2, n=512)
            for e in range(NP):
                ft = pf.tile([128, 1024], F16, tag="f")
                nc.sync.dma_start(ft[:], fc2t.ap()[:, e * 1024:(e + 1) * 1024])
                fts.append(ft)
            for e in range(NP):
                yt = py.tile([128, 512], F32, tag="y")
                # halves of partition dim hold chunks 2e and 2e+1
                nc.sync.dma_start(yt[:], ypb_g[e])
                yts.append(yt)
        with (
            tc.tile_pool(name="pg1", bufs=2, space="PSUM") as pg1,
            tc.tile_pool(name="pg2", bufs=2, space="PSUM") as pg2,
            tc.tile_pool(name="pa", bufs=2, space="PSUM") as pa,
            tc.tile_pool(name="px", bufs=2) as px,
            tc.tile_pool(name="ps1", bufs=2) as ps1,
            tc.tile_pool(name="ps2", bufs=2) as ps2,
        ):
            for t in range(S):
                h1p = ch0[:, 0:1] if t == 0 else H1T[:, t - 1:t]
                h2p = ch0[:, 1:2] if t == 0 else H2T[:, t - 1:t]
                # x_t = relu(Zp_t + M@h2 + v + fc1_b)
                xt = px.tile([128, 1], F16, tag="x")
                if t == 0:
                    dve.tensor_scalar(xt[:], Rt[:, 0:1], t0b[:], 0.0,
                                      op0=AluOpType.add, op1=AluOpType.max)
                else:
                    aps = pa.tile([128, 1], F32, tag="a")
                    mm(aps[:], MT16[:], h2p)
                    dve.tensor_scalar(xt[:], aps[:], ZpB[:, t:t + 1], 0.0,
                                      op0=AluOpType.add, op1=AluOpType.max)
                # LSTM1
                g1 = pg1.tile([128, 4], F32, tag="g1")
                for g in range(4):
                    mm(g1[:, g:g + 1], cw2[:, g * 128:(g + 1) * 128], h1p,
                       start=True, stop=False)
                    mm(g1[:, g:g + 1], cw1[:, g * 128:(g + 1) * 128], xt[:],
                       start=False, stop=True)
                s1 = ps1.tile([128, 6], F32, tag="s1")  # f i o g t1 tc
                act(s1[:, 0:1], g1[:, 0:1], AF.Sigmoid, bias=cb1[:, 0:1])
                act(s1[:, 1:2], g1[:, 1:2], AF.Sigmoid, bias=cb1[:, 1:2])
                act(s1[:, 2:3], g1[:, 2:3], AF.Sigmoid, bias=cb1[:, 2:3])
                act(s1[:, 3:4], g1[:, 3:4], AF.Tanh, bias=cb1[:, 3:4])
                dve.tensor_scalar_mul(s1[:, 4:5], s1[:, 3:4], s1[:, 1:2])
                dve.scalar_tensor_tensor(C1[:], C1[:], s1[:, 0:1], s1[:, 4:5],
                                         op0=AluOpType.mult, op1=AluOpType.add)
                act(s1[:, 5:6], C1[:], AF.Tanh)
                dve.tensor_scalar_mul(H1T[:, t:t + 1], s1[:, 5:6], s1[:, 2:3])
                # LSTM2
                g2 = pg2.tile([128, 4], F32, tag="g2")
                for g in range(4):
                    mm(g2[:, g:g + 1], cw4[:, g * 128:(g + 1) * 128], h2p,
                       start=True, stop=False)
                    mm(g2[:, g:g + 1], cw3[:, g * 128:(g + 1) * 128],
                       H1T[:, t:t + 1], start=False, stop=True)
                s2 = ps2.tile([128, 6], F32, tag="s2")
                act(s2[:, 0:1], g2[:, 0:1], AF.Sigmoid, bias=cb2[:, 0:1])
                act(s2[:, 1:2], g2[:, 1:2], AF.Sigmoid, bias=cb2[:, 1:2])
                act(s2[:, 2:3], g2[:, 2:3], AF.Sigmoid, bias=cb2[:, 2:3])
                act(s2[:, 3:4], g2[:, 3:4], AF.Tanh, bias=cb2[:, 3:4])
                dve.tensor_scalar_mul(s2[:, 4:5], s2[:, 3:4], s2[:, 1:2])
                dve.scalar_tensor_tensor(C2[:], C2[:], s2[:, 0:1], s2[:, 4:5],
                                         op0=AluOpType.mult, op1=AluOpType.add)
                act(s2[:, 5:6], C2[:], AF.Tanh)
                dve.tensor_scalar_mul(H2T[:, t:t + 1], s2[:, 5:6], s2[:, 2:3])

        # final states [h1f | c1f | h2f | c2f]
        dve.tensor_copy(so[:, 0:1], H1T[:, S - 1:S])
        dve.tensor_copy(so[:, 1:2], C1[:])
        dve.tensor_copy(so[:, 2:3], H2T[:, S - 1:S])
        dve.tensor_copy(so[:, 3:4], C2[:])
        nc.sync.dma_start(stout.ap(), so[:])

        # ---- phase 5: outs = H2 @ fc2.T (+ fc2_b + y, pre-summed on host) --
        if phases == "p1c":
            return
        with (
            tc.tile_pool(name="po", bufs=6, space="PSUM") as po,
            tc.tile_pool(name="pob", bufs=6) as pob,
        ):
            outp_g = outp.ap().rearrange("t (e dd n) -> e dd t n", dd=2, n=512)
            for e in range(NP):
                yt = yts[e]
                ob = pob.tile([128, 512], F32, tag="ob")
                for h in range(2):
                    ops = po.tile([S, 512], F32, tag="o")
                    mm(ops[:], H2T[:], fts[e][:, 512 * h:512 * (h + 1)])
                    sl_ob = ob[64 * h:64 * (h + 1), :]
                    sl_y = yt[64 * h:64 * (h + 1), :]
                    if (e + h) % 3:
                        dve.tensor_add(sl_ob, ops[:], sl_y)
                    else:
                        # GPSIMD cannot read PSUM: stage via ScalarE copy
                        nc.scalar.copy(sl_ob, ops[:])
                        nc.gpsimd.tensor_add(sl_ob, sl_ob, sl_y)
                nc.sync.dma_start(outp_g[e], ob[:])
        py_cm.__exit__(None, None, None)
        pf_cm.__exit__(None, None, None)


_COMPILED = {}


def _build(ncores=NCORES, phases="all"):
    key = (ncores, phases)
    if key in _COMPILED:
        return _COMPILED[key]
    nc = bacc.Bacc("TRN2", target_bir_lowering=False, debug=False,
                   num_devices=ncores)
    t_in = {}
    for name, shape, dt in [
        ("bigin", [128, NCH, CW], F16),
        ("fc2t", [128, KS], F16), ("ypb", [S, KS], F32),
        ("wih1t", [128, 512], F16), ("whh1t", [128, 512], F16),
        ("wih2t", [128, 512], F16), ("whh2t", [128, 512], F16),
        ("b1c", [128, 4], F32), ("b2c", [128, 4], F32), ("fc1b", [128, 1], F32),
        ("h016", [128, 2], F16), ("c0in", [128, 2], F32),
    ]:
        t_in[name] = nc.dram_tensor(name, shape, dt, kind="ExternalInput")
    t_out = {
        "outp": nc.dram_tensor("outp", [S, KS], F32, kind="ExternalOutput"),
        "stout": nc.dram_tensor("stout", [128, 4], F32, kind="ExternalOutput"),
    }
    with tile.TileContext(nc) as tc:
        _emit(nc, tc, t_in, t_out, ncores=ncores, phases=phases)
    nc.compile()
    _COMPILED[key] = (nc, t_in, t_out)
    return _COMPILED[key]


LAST_RESULTS = None


def prepare_in_maps(z, y, prev_gen, h1, c1, h2, c2,
                    fc1_w, fc1_b, w_ih1, w_hh1, b_ih1, b_hh1,
                    w_ih2, w_hh2, b_ih2, b_hh2, fc2_w, fc2_b):
    z = np.asarray(z, np.float32)
    y = np.asarray(y, np.float32)
    z_flat = z.reshape(S, NF)
    y_flat = y.reshape(S, NF)
    fc1_w = np.asarray(fc1_w, np.float32)
    fc2_w = np.asarray(fc2_w, np.float32)
    fc2_b = np.asarray(fc2_b, np.float32)
    prev_gen = np.asarray(prev_gen, np.float32)

    w1z = fc1_w[:, :NF]
    w1p = fc1_w[:, NF:]

    def pr(a, dt=np.float16):
        return np.ascontiguousarray(a).astype(dt)

    shared = {
        "wih1t": pr(np.asarray(w_ih1, np.float32)[_GIDX].T),
        "whh1t": pr(np.asarray(w_hh1, np.float32)[_GIDX].T),
        "wih2t": pr(np.asarray(w_ih2, np.float32)[_GIDX].T),
        "whh2t": pr(np.asarray(w_hh2, np.float32)[_GIDX].T),
        "b1c": pr((np.asarray(b_ih1, np.float32) + np.asarray(b_hh1, np.float32))[_GIDX].reshape(4, 128).T, np.float32),
        "b2c": pr((np.asarray(b_ih2, np.float32) + np.asarray(b_hh2, np.float32))[_GIDX].reshape(4, 128).T, np.float32),
        "fc1b": pr(np.asarray(fc1_b, np.float32).reshape(128, 1), np.float32),
        "h016": pr(np.concatenate([np.asarray(h1, np.float32).T,
                                   np.asarray(h2, np.float32).T], axis=1)),
        "c0in": pr(np.concatenate([np.asarray(c1, np.float32).T,
                                   np.asarray(c2, np.float32).T], axis=1), np.float32),
    }
    def perm(a):
        # [KS, w] -> [128, NCH, w] with [p, m, w] = a[m*128+p, w]
        return a.reshape(NCH, 128, a.shape[1]).transpose(1, 0, 2)

    in_maps = []
    for c in range(NCORES):
        sl = slice(c * KS, (c + 1) * KS)
        m = dict(shared)
        big = np.empty((128, NCH, CW), np.float16)
        big[:, :, 0:128] = perm(w1z[:, sl].T)
        big[:, :, 128:256] = perm(w1p[:, sl].T)
        big[:, :, 256:384] = perm(fc2_w[sl, :])
        big[:, :, 384:448] = perm(z_flat[:, sl].T)
        big[:, :, 448:450] = perm(np.stack([fc2_b[sl], prev_gen[0, sl]], axis=1))
        m["bigin"] = big
        m["fc2t"] = pr(fc2_w[sl, :].T)
        m["ypb"] = pr(y_flat[:, sl] + fc2_b[sl][None, :], np.float32)
        in_maps.append(m)
    return in_maps


def kernel(**inputs):
    global LAST_RESULTS
    import os
    nc, _, _ = _build()
    in_maps = prepare_in_maps(**inputs)
    trace = bool(int(os.environ.get("KERNEL_TRACE", "0")))
    res = bass_utils.run_bass_kernel_spmd(
        nc, in_maps, core_ids=list(range(NCORES)), trace=trace)
    LAST_RESULTS = res

    gen = np.concatenate([res.results[c]["outp"] for c in range(NCORES)],
                         axis=1).reshape(S, 3, 256, 256)
    st = res.results[0]["stout"]
    h1f = st[:, 0].reshape(1, HD).copy()
    c1f = st[:, 1].reshape(1, HD).copy()
    h2f = st[:, 2].reshape(1, HD).copy()
    c2f = st[:, 3].reshape(1, HD).copy()
    return (gen, h1f, c1f, h2f, c2f)
